# revision 22
# baseline (speedup 1.0000x reference)
"""Distributed 2-layer GCN (GCLEncoder) on 8 Trainium2 NeuronCores — Bass/Tile.

kernel(**inputs) takes the FULL inputs (x [100000,128] f32, W1 [128,64],
b1 [64], W2 [64,32], b2 [32], edge_index [2,1600000] i32) and returns the
FULL output z [100000, 32] f32.

v3 design (vs v2 baseline):
- Destination nodes sharded contiguously across 8 cores (12500 each, packed
  into 98 groups of <=128). Two-tier edge-count caps per (group, window)
  cell: groups 0..7 hold up to 640 edges/window (5 blocks), groups 8..97 up
  to 512 (4 blocks) -> 212,992 gather slots/layer vs 250,880 uniform.
- Per-layer node tables G (64 real bf16 features in 256B-strided rows,
  upper halves never written/read), exchanged via AllGather.
- Gathers: one dma_gather per (octant, window) chunk (32/layer); gpsimd
  descriptor generation is the kernel bottleneck (~8-9ns/row).
- Aggregation accumulates in PSUM across all 4 windows of an octant
  (no SBUF accumulator, no vector adds). Octants of ~12 groups keep
  PSUM under 16KB/partition.
- One-hot S built on DVE in one tensor_tensor per (group, window) cell
  ([128, nblk*128] is_equal vs iota), ~105ns/block.
- Self-loop injected via identity matmul from the SBUF-resident own-row
  stage; biases injected via rank-1 (sqrt(deg) (x) b) matmuls, so the
  finishes are pure scalar-engine activations (Relu/Copy with per-dst
  dinv scale) -- the vector engine never reads PSUM (avoids PE-PSUM port
  contention which made DVE ops 10-100x slower in v2).
- Layer 2 aggregates transposed (aggT [64h, 128d] PSUM; lhsT=rows,
  rhs=S), so the output projection needs no transpose:
  z = dinv * (aggT^T @ W2) + b2 via one matmul + scalar Copy.
- PSUM start=True resets a whole 2KB bank on HW (not just the addressed
  region), so group accumulators packed into shared bank tiles are
  zeroed once per octant by a full-bank K=1 matmul and every subsequent
  matmul accumulates with start=False (skip_group_check).
- AllGather outputs are Shared-addr-space DRAM (no ring copy).
"""

from dataclasses import dataclass

import numpy as np
import ml_dtypes

import concourse.bass as bass
import concourse.tile as tile
import concourse.bacc as bacc
from concourse import bass_utils, mybir
from concourse.masks import make_identity

F32 = mybir.dt.float32
BF16 = mybir.dt.bfloat16
I16 = mybir.dt.int16
P = 128
NWIN = 4
NOCT = 8
OVG = 4          # overflow groups (5-block cells); rest are 4-block
CAP_HI = 640
CAP_LO = 512


@dataclass(frozen=True)
class Cfg:
    n_nodes: int
    din: int
    dh: int
    dout: int
    C: int
    NG: int
    nblk: tuple  # per-cell block count, cell = g * NWIN + w

    @property
    def npc(self):
        return self.n_nodes // self.C

    @property
    def npcp(self):
        return self.NG * P

    @property
    def NB(self):
        return int(sum(self.nblk))

    @property
    def oct_groups(self):
        base = self.NG // NOCT
        rem = self.NG % NOCT
        sizes = [base + (1 if o < rem else 0) for o in range(NOCT)]
        out, s = [], 0
        for sz in sizes:
            out.append((s, sz))
            s += sz
        return tuple(out)

    @property
    def winrows(self):
        return self.npcp * self.C // NWIN

    def cell_nblk(self, g, w):
        return self.nblk[g * NWIN + w]

    @property
    def block_col(self):
        """column offset of each cell's blocks in dstRel, ordered
        (oct, w, g-in-oct, j)."""
        col = {}
        b = 0
        for o, (g0, gn) in enumerate(self.oct_groups):
            for w in range(NWIN):
                for g in range(g0, g0 + gn):
                    col[(g, w)] = b
                    b += self.cell_nblk(g, w)
        return col

    @property
    def chunk_info(self):
        """per (oct, w): (block col offset, nblk total, idx col offset)."""
        out = {}
        b = 0
        for o, (g0, gn) in enumerate(self.oct_groups):
            for w in range(NWIN):
                nb = sum(self.cell_nblk(g, w) for g in range(g0, g0 + gn))
                out[(o, w)] = (b, nb, b * 8)
                b += nb
        return out


def _balance_groups(degw_local, NG, caps):
    """Assign npc nodes to NG groups (<=128 nodes each) s.t. per-(g,w)
    edge counts stay under caps[g]. Greedy + repair."""
    npc, W = degw_local.shape
    order = np.argsort(-degw_local.sum(axis=1), kind="stable")
    sums = np.zeros((NG, W), dtype=np.int64)
    cnts = np.zeros(NG, dtype=np.int64)
    binof = np.full(npc, -1, dtype=np.int64)
    capv = caps[:, None]

    for nidx in order:
        d = degw_local[nidx][None, :]
        cand = sums + d
        over = np.maximum(cand - capv, 0).sum(axis=1)
        # prefer no violation, then lowest relative fill
        score = over * 1e6 + (cand / capv).max(axis=1)
        score[cnts >= P] = np.inf
        b = int(np.argmin(score))
        binof[nidx] = b
        cnts[b] += 1
        sums[b] += degw_local[nidx]

    nodes_of = [list(np.where(binof == b)[0]) for b in range(NG)]
    rng = np.random.default_rng(12345)
    for it in range(60000):
        viol = np.maximum(sums - capv, 0)
        tot_v = viol.sum()
        if tot_v == 0:
            break
        g, w = np.unravel_index(int(np.argmax(viol)), viol.shape)
        # try moving a node out of g into a group with room (and node space)
        du = degw_local[nodes_of[g]]
        u_order = np.argsort(-du[:, w])[:16]
        room = capv - sums  # [NG, W]
        done = False
        for ui in u_order:
            d_u = du[ui]
            fits = (room >= d_u[None, :]).all(axis=1) & (cnts < P)
            fits[g] = False
            if fits.any():
                cands = np.where(fits)[0]
                gp = int(cands[int(np.argmin((sums[cands] / capv[cands]).max(axis=1)))])
                u = nodes_of[g][ui]
                nodes_of[g].pop(ui)
                nodes_of[gp].append(u)
                binof[u] = gp
                sums[g] -= d_u
                sums[gp] += d_u
                cnts[g] -= 1
                cnts[gp] += 1
                done = True
                break
        if done:
            continue
        # swap: node u in g with node v in gp s.t. violations shrink
        best = None
        ui = int(u_order[0])
        d_u = degw_local[nodes_of[g][ui]]
        for gp in rng.permutation(NG)[:24]:
            gp = int(gp)
            if gp == g:
                continue
            dv = degw_local[nodes_of[gp]]
            ng = sums[g] - d_u[None, :] + dv
            ngp = sums[gp] + d_u[None, :] - dv
            v_new = (np.maximum(ng - capv[g], 0).sum(axis=1)
                     + np.maximum(ngp - capv[gp], 0).sum(axis=1))
            vi = int(np.argmin(v_new))
            base_v = viol[g].sum() + viol[gp].sum()
            if v_new[vi] < base_v and (best is None or v_new[vi] < best[0]):
                best = (v_new[vi], vi, gp)
        if best is None:
            continue
        _, vi, gp = best
        u = nodes_of[g][ui]
        v = nodes_of[gp][vi]
        sums[g] += degw_local[v] - degw_local[u]
        sums[gp] += degw_local[u] - degw_local[v]
        binof[u], binof[v] = gp, g
        nodes_of[g][ui] = v
        nodes_of[gp][vi] = u
    ok = (np.maximum(sums - capv, 0).sum() == 0)
    return binof, ok


def build_schedule(edge_index: np.ndarray, n_nodes: int, C: int):
    src = np.asarray(edge_index[0], dtype=np.int64)
    dst = np.asarray(edge_index[1], dtype=np.int64)
    npc = n_nodes // C
    NG = -(-npc // P)
    npcp = NG * P

    deg = 1.0 + np.bincount(dst, minlength=n_nodes).astype(np.float32)
    owner = dst // npc
    ewin = (src // npc) // (C // NWIN)
    degw = np.zeros((n_nodes, NWIN), dtype=np.int64)
    np.add.at(degw, (dst, ewin), 1)

    caps = np.full(NG, CAP_LO, dtype=np.int64)
    caps[:OVG] = CAP_HI

    pos_of_node = np.zeros((C, npc), dtype=np.int64)
    node_of_pos = np.full((C, npcp), -1, dtype=np.int64)
    cell_cnt = np.zeros((C, NG, NWIN), dtype=np.int64)
    for c in range(C):
        degw_c = degw[c * npc : (c + 1) * npc]
        binof, ok = _balance_groups(degw_c, NG, caps)
        order = np.argsort(binof, kind="stable")
        counts = np.bincount(binof, minlength=NG)
        starts = np.cumsum(counts) - counts
        slot = np.zeros(npc, dtype=np.int64)
        slot[order] = np.arange(npc) - starts[binof[order]]
        pos = binof * P + slot
        pos_of_node[c] = pos
        node_of_pos[c, pos] = np.arange(npc)
        for w in range(NWIN):
            np.add.at(cell_cnt[c, :, w], binof, degw_c[:, w])

    # per-cell blocks = max over cores (SPMD shares one NEFF)
    nblk = tuple(
        int(-(-cell_cnt[:, g, w].max() // P)) for g in range(NG) for w in range(NWIN)
    )
    cfg = Cfg(n_nodes=n_nodes, din=128, dh=64, dout=32, C=C, NG=NG, nblk=nblk)
    NB = cfg.NB
    block_col = cfg.block_col
    chunk_info = cfg.chunk_info
    winrows = cfg.winrows

    growp = (np.arange(n_nodes) // npc) * npcp + pos_of_node[
        np.arange(n_nodes) // npc, np.arange(n_nodes) % npc
    ]
    s_row = growp[src]
    s_win = ewin  # window = owner(src) pair, independent of balance
    s_idx = s_row % winrows

    d_pos = pos_of_node[owner, dst % npc]
    d_grp = d_pos // P
    d_rel = d_pos % P

    # rank within (owner, d_grp, s_win)
    flat = ((owner * NG + d_grp) * NWIN + s_win).astype(np.int64)
    order = np.lexsort((s_win, d_grp, owner))
    counts = np.bincount(flat, minlength=C * NG * NWIN)
    starts = np.cumsum(counts) - counts
    rank = np.arange(len(src)) - starts[flat[order]]

    cell_nblk_arr = np.array(cfg.nblk, dtype=np.int64).reshape(NG, NWIN)
    col0 = np.zeros((NG, NWIN), dtype=np.int64)
    for (g, w), b in block_col.items():
        col0[g, w] = b
    GIDXCOLS = NB * 8

    data = []
    for c in range(C):
        m = owner[order] == c
        es = order[m]
        rk = rank[m]
        g = d_grp[es]
        w = s_win[es]
        assert (rk < cell_nblk_arr[g, w] * P).all(), "cell overflow"
        j = rk // P
        s = rk % P
        col_b = col0[g, w] + j

        gidx = np.zeros(NB * P, dtype=np.int16)
        gidx[col_b * P + s] = s_idx[es].astype(np.int16)
        dst_rel = np.full((P, NB), -1.0, dtype=np.float32)
        dst_rel[s, col_b] = d_rel[es].astype(np.float32)

        # wrap16 per chunk
        gidx_sb = np.zeros((16, GIDXCOLS), dtype=np.int16)
        for (o, wc), (b0, nb, c0) in chunk_info.items():
            a = gidx[b0 * P : (b0 + nb) * P].reshape(nb * 8, 16)
            gidx_sb[:, c0 : c0 + nb * 8] = a.T
        gidx_sb = np.tile(gidx_sb, (8, 1))

        deg_nm = np.ones(npcp, np.float32)
        valid = node_of_pos[c] >= 0
        deg_nm[valid] = deg[c * npc + node_of_pos[c][valid]]

        data.append(
            {
                "gidx": gidx_sb,
                "dst_rel": dst_rel.astype(ml_dtypes.bfloat16),
                "deg_nm": deg_nm,
                "pos_of_node": pos_of_node[c],
            }
        )
    return cfg, data


def build_inputs(cfg: Cfg, x, W1, b1, W2, b2, sched):
    C, npc, npcp = cfg.C, cfg.npc, cfg.npcp
    x = np.asarray(x, dtype=np.float32)
    in_maps = []
    for c in range(C):
        xT = np.zeros((P, npcp), dtype=np.float32)
        pos = sched[c]["pos_of_node"]
        xT[:, pos] = x[c * npc : (c + 1) * npc].T
        deg = sched[c]["deg_nm"]
        dinv = (1.0 / np.sqrt(deg)).astype(np.float32)
        in_maps.append(
            {
                "xT": xT.astype(ml_dtypes.bfloat16),
                "W1in": np.asarray(W1, np.float32).astype(ml_dtypes.bfloat16),
                "W2in": np.asarray(W2, np.float32),
                "b1in": np.asarray(b1, np.float32)[None, :],
                "b2in": np.asarray(b2, np.float32)[None, :],
                "gIdx": sched[c]["gidx"],
                "dstRel": sched[c]["dst_rel"],
                "dinvNM": dinv.reshape(cfg.NG, P).T.copy(),   # [128, NG]
                "sqdQ": np.sqrt(deg).astype(np.float32)[None, :],  # [1, npcp]
            }
        )
    return in_maps


def build_nc(cfg: Cfg):
    C, DH, DOUT = cfg.C, cfg.dh, cfg.dout
    NG, NB, npcp, winrows = cfg.NG, cfg.NB, cfg.npcp, cfg.winrows
    oct_groups = cfg.oct_groups
    block_col = cfg.block_col
    chunk_info = cfg.chunk_info
    GIDXCOLS = NB * 8
    max_chunk_nb = max(nb for (_, nb, _) in chunk_info.values())
    max_cell_nb = max(cfg.nblk)

    nc = bacc.Bacc("TRN2", target_bir_lowering=False, debug=False, num_devices=C)

    xT = nc.dram_tensor("xT", [P, npcp], BF16, kind="ExternalInput").ap()
    W1in = nc.dram_tensor("W1in", [P, DH], BF16, kind="ExternalInput").ap()
    W2in = nc.dram_tensor("W2in", [DH, DOUT], F32, kind="ExternalInput").ap()
    b1in = nc.dram_tensor("b1in", [1, DH], F32, kind="ExternalInput").ap()
    b2in = nc.dram_tensor("b2in", [1, DOUT], F32, kind="ExternalInput").ap()
    gIdx = nc.dram_tensor("gIdx", [P, GIDXCOLS], I16, kind="ExternalInput").ap()
    dstRel = nc.dram_tensor("dstRel", [P, NB], BF16, kind="ExternalInput").ap()
    dinvNM = nc.dram_tensor("dinvNM", [P, NG], F32, kind="ExternalInput").ap()
    sqdQ = nc.dram_tensor("sqdQ", [1, npcp], F32, kind="ExternalInput").ap()
    z = nc.dram_tensor("z", [npcp, DOUT], F32, kind="ExternalOutput").ap()

    with tile.TileContext(nc) as tc:
        with (
            tc.tile_pool(name="const", bufs=1) as cpool,
            tc.tile_pool(name="work", bufs=1) as wpool,
            tc.tile_pool(name="psum", bufs=1, space="PSUM") as ppool,
            tc.tile_pool(name="dram", bufs=1, space="DRAM") as dpool,
        ):
            W1sb = cpool.tile([P, DH], BF16)
            nc.sync.dma_start(W1sb[:], W1in[:])
            W2sb = cpool.tile([DH, DOUT], F32)
            nc.sync.dma_start(W2sb[:], W2in[:])
            b1sb = cpool.tile([1, DH], F32)
            nc.sync.dma_start(b1sb[:], b1in[:])
            b2sb = cpool.tile([1, DOUT], F32)
            nc.sync.dma_start(b2sb[:], b2in[:])
            dinv = cpool.tile([P, NG], F32)
            nc.sync.dma_start(dinv[:], dinvNM[:])
            sqd = cpool.tile([1, npcp], F32)
            nc.sync.dma_start(sqd[:], sqdQ[:])
            dst_sb = cpool.tile([P, NB], BF16)
            nc.sync.dma_start(dst_sb[:], dstRel[:])
            gidx_sb = cpool.tile([P, GIDXCOLS], I16)
            nc.sync.dma_start(gidx_sb[:], gIdx[:])

            zrow = cpool.tile([1, 512], F32)
            nc.vector.memset(zrow[:], 0.0)
            identb = cpool.tile([P, P], BF16)
            make_identity(nc, identb[:])
            iota_i = cpool.tile([P, P], mybir.dt.int32)
            nc.gpsimd.iota(iota_i[:], pattern=[[1, P]], base=0, channel_multiplier=0)
            iota_b = cpool.tile([P, P], BF16)
            nc.vector.tensor_copy(iota_b[:], iota_i[:])

            stage1 = cpool.tile([P, NG * DH], BF16)   # G1 own rows (table vals)
            stage2 = cpool.tile([P, NG * DH], BF16)   # G2 own rows

            G1s = dpool.tile([npcp, P], BF16)
            G1f = dpool.tile([C * npcp, P], BF16, addr_space="Shared")
            G2s = dpool.tile([npcp, P], BF16)
            G2f = dpool.tile([C * npcp, P], BF16, addr_space="Shared")
            rg = [list(range(C))]

            # ---- layer 1 dense: stage1 = dinv * (x @ W1) (bf16), to G1s ----
            xt_all = cpool.tile([P, npcp], BF16)
            nc.sync.dma_start(xt_all[:], xT[:])
            for g in range(NG):
                ps = ppool.tile([P, DH], F32, tag="pdense", bufs=1, space="PSUM")
                nc.tensor.matmul(
                    ps[:], lhsT=xt_all[:, g * P : (g + 1) * P], rhs=W1sb[:],
                    start=True, stop=True,
                )
                sl = stage1[:, g * DH : (g + 1) * DH]
                nc.scalar.activation(
                    sl, ps[:], mybir.ActivationFunctionType.Copy,
                    scale=dinv[:, g : g + 1],
                )
                nc.sync.dma_start(G1s[g * P : (g + 1) * P, 0:DH], sl)

            nc.gpsimd.collective_compute(
                "AllGather", mybir.AluOpType.bypass, replica_groups=rg,
                ins=[G1s[:]], outs=[G1f[:]],
            )

            # last nonempty (w, j) per group, for the PSUM stop flag
            last_wj = {}
            for g in range(NG):
                last_wj[g] = None
                for w in range(NWIN):
                    if cfg.cell_nblk(g, w) > 0:
                        last_wj[g] = (w, cfg.cell_nblk(g, w) - 1)

            def run_layer(Gf, layer):
                for o, (g0, gn) in enumerate(oct_groups):
                    # pack group accumulators into bank-sized PSUM tiles
                    psg_of = {}
                    if layer == 1:
                        nbank = -(-gn // 8)
                        banks = [
                            ppool.tile(
                                [P, 512], F32, tag=f"ps1b_{k}", bufs=1,
                                space="PSUM", name=f"psg1_{o}_{k}",
                            )
                            for k in range(nbank)
                        ]
                        for g in range(g0, g0 + gn):
                            i = g - g0
                            psg_of[g] = banks[i // 8][:, (i % 8) * DH : (i % 8 + 1) * DH]
                        for k in range(nbank):
                            # start=True resets the whole PSUM bank on HW, so
                            # zero each bank once and accumulate into slices
                            nc.tensor.matmul(
                                banks[k][:], lhsT=zrow[:, 0:P], rhs=zrow[:],
                                start=True, stop=False, skip_group_check=True,
                            )
                    else:
                        nbank = -(-gn // 4)
                        banks = [
                            ppool.tile(
                                [DH, 512], F32, tag=f"ps2b_{k}", bufs=1,
                                space="PSUM", name=f"psg2_{o}_{k}",
                            )
                            for k in range(nbank)
                        ]
                        for g in range(g0, g0 + gn):
                            i = g - g0
                            psg_of[g] = banks[i // 4][:, (i % 4) * P : (i % 4 + 1) * P]
                        for k in range(nbank):
                            nc.tensor.matmul(
                                banks[k][:], lhsT=zrow[:, 0:DH], rhs=zrow[:],
                                start=True, stop=False, skip_group_check=True,
                            )
                    for w in range(NWIN):
                        b0, nb, c0 = chunk_info[(o, w)]
                        if nb == 0:
                            continue
                        rows = wpool.tile(
                            [P, max_chunk_nb, P], BF16, tag="rows", bufs=2
                        )
                        nc.gpsimd.dma_gather(
                            out_ap=rows[:, 0:nb, :],
                            in_ap=Gf[w * winrows : (w + 1) * winrows, :],
                            idxs_ap=gidx_sb[:, c0 : c0 + nb * 8],
                            num_idxs=nb * P,
                            num_idxs_reg=nb * P,
                            elem_size=P,
                            single_packet=False,
                        )
                        jj = 0
                        for g in range(g0, g0 + gn):
                            cnb = cfg.cell_nblk(g, w)
                            bcol = block_col[(g, w)]
                            psg = psg_of[g]
                            first_w = (w == 0) or all(
                                cfg.cell_nblk(g, w2) == 0 for w2 in range(w)
                            )
                            if first_w:
                                # self-loop + bias injection opens the group
                                only = last_wj[g] is None
                                if layer == 1:
                                    nc.tensor.matmul(
                                        psg, lhsT=identb[:],
                                        rhs=stage1[:, g * DH : (g + 1) * DH],
                                        start=False, stop=False,
                                        skip_group_check=True,
                                    )
                                    nc.tensor.matmul(
                                        psg,
                                        lhsT=sqd[:, g * P : (g + 1) * P],
                                        rhs=b1sb[:],
                                        start=False, stop=only,
                                        skip_group_check=True,
                                    )
                                else:
                                    nc.tensor.matmul(
                                        psg,
                                        lhsT=stage2[:, g * DH : (g + 1) * DH],
                                        rhs=identb[:],
                                        start=False, stop=only,
                                        skip_group_check=True,
                                    )
                            if cnb == 0:
                                continue
                            S = wpool.tile(
                                [P, max_cell_nb, P], BF16, tag="S", bufs=4
                            )
                            nc.vector.tensor_tensor(
                                out=S[:, 0:cnb, :],
                                in0=dst_sb[:, bcol : bcol + cnb]
                                .unsqueeze(2).to_broadcast([P, cnb, P]),
                                in1=iota_b[:].unsqueeze(1).to_broadcast([P, cnb, P]),
                                op=mybir.AluOpType.is_equal,
                            )
                            for j in range(cnb):
                                last = last_wj[g] == (w, j)
                                if layer == 1:
                                    nc.tensor.matmul(
                                        psg, lhsT=S[:, j, :],
                                        rhs=rows[:, jj + j, 0:DH],
                                        start=False, stop=last,
                                        skip_group_check=True,
                                    )
                                else:
                                    nc.tensor.matmul(
                                        psg, lhsT=rows[:, jj + j, 0:DH],
                                        rhs=S[:, j, :],
                                        start=False, stop=last,
                                        skip_group_check=True,
                                    )
                            jj += cnb
                    # finish the octant's groups
                    for g in range(g0, g0 + gn):
                        psg = psg_of[g]
                        if layer == 1:
                            tmp = wpool.tile([P, DH], F32, tag="f1tmp", bufs=3)
                            nc.scalar.activation(
                                tmp[:], psg, mybir.ActivationFunctionType.Relu,
                                scale=dinv[:, g : g + 1],
                            )
                            sl = stage2[:, g * DH : (g + 1) * DH]
                            nc.scalar.activation(
                                sl, tmp[:], mybir.ActivationFunctionType.Copy,
                                scale=dinv[:, g : g + 1],
                            )
                            nc.sync.dma_start(G2s[g * P : (g + 1) * P, 0:DH], sl)

                        else:
                            aT = wpool.tile([DH, P], F32, tag="aT", bufs=3)
                            nc.scalar.activation(
                                aT[:], psg, mybir.ActivationFunctionType.Copy,
                            )
                            zp = ppool.tile(
                                [P, DOUT], F32, tag="zp", bufs=1, space="PSUM"
                            )
                            nc.tensor.matmul(
                                zp[:], lhsT=aT[:], rhs=W2sb[:],
                                start=True, stop=False,
                            )
                            nc.tensor.matmul(
                                zp[:], lhsT=sqd[:, g * P : (g + 1) * P],
                                rhs=b2sb[:], start=False, stop=True,
                            )
                            zs = wpool.tile([P, DOUT], F32, tag="zs", bufs=3)
                            nc.scalar.activation(
                                zs[:], zp[:], mybir.ActivationFunctionType.Copy,
                                scale=dinv[:, g : g + 1],
                            )
                            nc.sync.dma_start(z[g * P : (g + 1) * P, :], zs[:])

            run_layer(G1f, 1)

            nc.gpsimd.collective_compute(
                "AllGather", mybir.AluOpType.bypass, replica_groups=rg,
                ins=[G2s[:]], outs=[G2f[:]],
            )

            run_layer(G2f, 2)

    nc.compile()
    return nc


N_CORES = 8
_NC_CACHE = {}
_SCHED_CACHE = {}


def _cached_nc(cfg):
    if cfg not in _NC_CACHE:
        _NC_CACHE[cfg] = build_nc(cfg)
    return _NC_CACHE[cfg]


def _cached_schedule(edge_index, n):
    ei = np.ascontiguousarray(edge_index)
    key = (ei.shape, n, hash(ei.tobytes()))
    if key not in _SCHED_CACHE:
        _SCHED_CACHE[key] = build_schedule(ei, n, N_CORES)
    return _SCHED_CACHE[key]


def kernel(x, W1, b1, W2, b2, edge_index):
    x = np.asarray(x)
    n = x.shape[0]
    cfg, sched = _cached_schedule(np.asarray(edge_index), n)
    in_maps = build_inputs(cfg, x, W1, b1, W2, b2, sched)
    nc = _cached_nc(cfg)
    res = bass_utils.run_bass_kernel_spmd(nc, in_maps, core_ids=list(range(N_CORES)))
    z = np.concatenate(
        [res.results[c]["z"][sched[c]["pos_of_node"]] for c in range(N_CORES)], axis=0
    )
    return z.astype(np.float32)


# revision 23
# speedup vs baseline: 1.2102x; 1.2102x over previous
"""Distributed 2-layer GCN (GCLEncoder) on 8 Trainium2 NeuronCores — Bass/Tile.

kernel(**inputs) takes the FULL inputs (x [100000,128] f32, W1 [128,64],
b1 [64], W2 [64,32], b2 [32], edge_index [2,1600000] i32) and returns the
FULL output z [100000, 32] f32.

v3 design (vs v2 baseline):
- Destination nodes sharded contiguously across 8 cores (12500 each, packed
  into 98 groups of <=128). Two-tier edge-count caps per (group, window)
  cell: groups 0..7 hold up to 640 edges/window (5 blocks), groups 8..97 up
  to 512 (4 blocks) -> 212,992 gather slots/layer vs 250,880 uniform.
- Per-layer node tables G (64 real bf16 features in 256B-strided rows,
  upper halves never written/read), exchanged via AllGather.
- Gathers: one dma_gather per (octant, window) chunk (32/layer); gpsimd
  descriptor generation is the kernel bottleneck (~8-9ns/row).
- Aggregation accumulates in PSUM across all 4 windows of an octant
  (no SBUF accumulator, no vector adds). Octants of ~12 groups keep
  PSUM under 16KB/partition.
- One-hot S built on DVE in one tensor_tensor per (group, window) cell
  ([128, nblk*128] is_equal vs iota), ~105ns/block.
- Self-loop injected via identity matmul from the SBUF-resident own-row
  stage; biases injected via rank-1 (sqrt(deg) (x) b) matmuls, so the
  finishes are pure scalar-engine activations (Relu/Copy with per-dst
  dinv scale) -- the vector engine never reads PSUM (avoids PE-PSUM port
  contention which made DVE ops 10-100x slower in v2).
- Layer 2 aggregates transposed (aggT [64h, 128d] PSUM; lhsT=rows,
  rhs=S), so the output projection needs no transpose:
  z = dinv * (aggT^T @ W2) + b2 via one matmul + scalar Copy.
- PSUM start=True resets a whole 2KB bank on HW (not just the addressed
  region), so group accumulators packed into shared bank tiles are
  zeroed once per octant by a full-bank K=1 matmul and every subsequent
  matmul accumulates with start=False (skip_group_check).
- AllGather outputs are Shared-addr-space DRAM (no ring copy).
"""

from dataclasses import dataclass

import numpy as np
import ml_dtypes

import concourse.bass as bass
import concourse.tile as tile
import concourse.bacc as bacc
from concourse import bass_utils, mybir
from concourse.masks import make_identity

F32 = mybir.dt.float32
BF16 = mybir.dt.bfloat16
I16 = mybir.dt.int16
P = 128
NWIN = 4
NOCT = 8
OVG = 4          # overflow groups (5-block cells); rest are 4-block
CAP_HI = 640
CAP_LO = 512


@dataclass(frozen=True)
class Cfg:
    n_nodes: int
    din: int
    dh: int
    dout: int
    C: int
    NG: int
    nblk: tuple  # per-cell block count, cell = g * NWIN + w

    @property
    def npc(self):
        return self.n_nodes // self.C

    @property
    def npcp(self):
        return self.NG * P

    @property
    def NB(self):
        return int(sum(self.nblk))

    @property
    def oct_groups(self):
        base = self.NG // NOCT
        rem = self.NG % NOCT
        sizes = [base + (1 if o < rem else 0) for o in range(NOCT)]
        out, s = [], 0
        for sz in sizes:
            out.append((s, sz))
            s += sz
        return tuple(out)

    @property
    def winrows(self):
        return self.npcp * self.C // NWIN

    def cell_nblk(self, g, w):
        return self.nblk[g * NWIN + w]

    @property
    def block_col(self):
        """column offset of each cell's blocks in dstRel, ordered
        (oct, w, g-in-oct, j)."""
        col = {}
        b = 0
        for o, (g0, gn) in enumerate(self.oct_groups):
            for w in range(NWIN):
                for g in range(g0, g0 + gn):
                    col[(g, w)] = b
                    b += self.cell_nblk(g, w)
        return col

    @property
    def chunk_info(self):
        """per (oct, w): (block col offset, nblk total, idx col offset)."""
        out = {}
        b = 0
        for o, (g0, gn) in enumerate(self.oct_groups):
            for w in range(NWIN):
                nb = sum(self.cell_nblk(g, w) for g in range(g0, g0 + gn))
                out[(o, w)] = (b, nb, b * 8)
                b += nb
        return out


def _balance_groups(degw_local, NG, caps):
    """Assign npc nodes to NG groups (<=128 nodes each) s.t. per-(g,w)
    edge counts stay under caps[g]. Greedy + repair."""
    npc, W = degw_local.shape
    order = np.argsort(-degw_local.sum(axis=1), kind="stable")
    sums = np.zeros((NG, W), dtype=np.int64)
    cnts = np.zeros(NG, dtype=np.int64)
    binof = np.full(npc, -1, dtype=np.int64)
    capv = caps[:, None]

    for nidx in order:
        d = degw_local[nidx][None, :]
        cand = sums + d
        over = np.maximum(cand - capv, 0).sum(axis=1)
        # prefer no violation, then lowest relative fill
        score = over * 1e6 + (cand / capv).max(axis=1)
        score[cnts >= P] = np.inf
        b = int(np.argmin(score))
        binof[nidx] = b
        cnts[b] += 1
        sums[b] += degw_local[nidx]

    nodes_of = [list(np.where(binof == b)[0]) for b in range(NG)]
    rng = np.random.default_rng(12345)
    for it in range(60000):
        viol = np.maximum(sums - capv, 0)
        tot_v = viol.sum()
        if tot_v == 0:
            break
        g, w = np.unravel_index(int(np.argmax(viol)), viol.shape)
        # try moving a node out of g into a group with room (and node space)
        du = degw_local[nodes_of[g]]
        u_order = np.argsort(-du[:, w])[:16]
        room = capv - sums  # [NG, W]
        done = False
        for ui in u_order:
            d_u = du[ui]
            fits = (room >= d_u[None, :]).all(axis=1) & (cnts < P)
            fits[g] = False
            if fits.any():
                cands = np.where(fits)[0]
                gp = int(cands[int(np.argmin((sums[cands] / capv[cands]).max(axis=1)))])
                u = nodes_of[g][ui]
                nodes_of[g].pop(ui)
                nodes_of[gp].append(u)
                binof[u] = gp
                sums[g] -= d_u
                sums[gp] += d_u
                cnts[g] -= 1
                cnts[gp] += 1
                done = True
                break
        if done:
            continue
        # swap: node u in g with node v in gp s.t. violations shrink
        best = None
        ui = int(u_order[0])
        d_u = degw_local[nodes_of[g][ui]]
        for gp in rng.permutation(NG)[:24]:
            gp = int(gp)
            if gp == g:
                continue
            dv = degw_local[nodes_of[gp]]
            ng = sums[g] - d_u[None, :] + dv
            ngp = sums[gp] + d_u[None, :] - dv
            v_new = (np.maximum(ng - capv[g], 0).sum(axis=1)
                     + np.maximum(ngp - capv[gp], 0).sum(axis=1))
            vi = int(np.argmin(v_new))
            base_v = viol[g].sum() + viol[gp].sum()
            if v_new[vi] < base_v and (best is None or v_new[vi] < best[0]):
                best = (v_new[vi], vi, gp)
        if best is None:
            continue
        _, vi, gp = best
        u = nodes_of[g][ui]
        v = nodes_of[gp][vi]
        sums[g] += degw_local[v] - degw_local[u]
        sums[gp] += degw_local[u] - degw_local[v]
        binof[u], binof[v] = gp, g
        nodes_of[g][ui] = v
        nodes_of[gp][vi] = u
    ok = (np.maximum(sums - capv, 0).sum() == 0)
    return binof, ok


def build_schedule(edge_index: np.ndarray, n_nodes: int, C: int):
    src = np.asarray(edge_index[0], dtype=np.int64)
    dst = np.asarray(edge_index[1], dtype=np.int64)
    npc = n_nodes // C
    NG = -(-npc // P)
    npcp = NG * P

    deg = 1.0 + np.bincount(dst, minlength=n_nodes).astype(np.float32)
    owner = dst // npc
    ewin = (src // npc) // (C // NWIN)
    degw = np.zeros((n_nodes, NWIN), dtype=np.int64)
    np.add.at(degw, (dst, ewin), 1)

    caps = np.full(NG, CAP_LO, dtype=np.int64)
    caps[:OVG] = CAP_HI

    pos_of_node = np.zeros((C, npc), dtype=np.int64)
    node_of_pos = np.full((C, npcp), -1, dtype=np.int64)
    cell_cnt = np.zeros((C, NG, NWIN), dtype=np.int64)
    for c in range(C):
        degw_c = degw[c * npc : (c + 1) * npc]
        binof, ok = _balance_groups(degw_c, NG, caps)
        order = np.argsort(binof, kind="stable")
        counts = np.bincount(binof, minlength=NG)
        starts = np.cumsum(counts) - counts
        slot = np.zeros(npc, dtype=np.int64)
        slot[order] = np.arange(npc) - starts[binof[order]]
        pos = binof * P + slot
        pos_of_node[c] = pos
        node_of_pos[c, pos] = np.arange(npc)
        for w in range(NWIN):
            np.add.at(cell_cnt[c, :, w], binof, degw_c[:, w])

    # per-cell blocks = max over cores (SPMD shares one NEFF)
    nblk = tuple(
        int(-(-cell_cnt[:, g, w].max() // P)) for g in range(NG) for w in range(NWIN)
    )
    cfg = Cfg(n_nodes=n_nodes, din=128, dh=64, dout=32, C=C, NG=NG, nblk=nblk)
    NB = cfg.NB
    block_col = cfg.block_col
    chunk_info = cfg.chunk_info
    winrows = cfg.winrows

    growp = (np.arange(n_nodes) // npc) * npcp + pos_of_node[
        np.arange(n_nodes) // npc, np.arange(n_nodes) % npc
    ]
    s_row = growp[src]
    s_win = ewin  # window = owner(src) pair, independent of balance
    s_idx = s_row % winrows

    d_pos = pos_of_node[owner, dst % npc]
    d_grp = d_pos // P
    d_rel = d_pos % P

    # rank within (owner, d_grp, s_win)
    flat = ((owner * NG + d_grp) * NWIN + s_win).astype(np.int64)
    order = np.lexsort((s_win, d_grp, owner))
    counts = np.bincount(flat, minlength=C * NG * NWIN)
    starts = np.cumsum(counts) - counts
    rank = np.arange(len(src)) - starts[flat[order]]

    cell_nblk_arr = np.array(cfg.nblk, dtype=np.int64).reshape(NG, NWIN)
    col0 = np.zeros((NG, NWIN), dtype=np.int64)
    for (g, w), b in block_col.items():
        col0[g, w] = b
    GIDXCOLS = NB * 8

    data = []
    for c in range(C):
        m = owner[order] == c
        es = order[m]
        rk = rank[m]
        g = d_grp[es]
        w = s_win[es]
        assert (rk < cell_nblk_arr[g, w] * P).all(), "cell overflow"
        j = rk // P
        s = rk % P
        col_b = col0[g, w] + j

        gidx = np.zeros(NB * P, dtype=np.int16)
        gidx[col_b * P + s] = s_idx[es].astype(np.int16)
        dst_rel = np.full((P, NB), -1.0, dtype=np.float32)
        dst_rel[s, col_b] = d_rel[es].astype(np.float32)

        # wrap16 per chunk
        gidx_sb = np.zeros((16, GIDXCOLS), dtype=np.int16)
        for (o, wc), (b0, nb, c0) in chunk_info.items():
            a = gidx[b0 * P : (b0 + nb) * P].reshape(nb * 8, 16)
            gidx_sb[:, c0 : c0 + nb * 8] = a.T
        gidx_sb = np.tile(gidx_sb, (8, 1))

        deg_nm = np.ones(npcp, np.float32)
        valid = node_of_pos[c] >= 0
        deg_nm[valid] = deg[c * npc + node_of_pos[c][valid]]

        data.append(
            {
                "gidx": gidx_sb,
                "dst_rel": dst_rel.astype(ml_dtypes.bfloat16),
                "deg_nm": deg_nm,
                "pos_of_node": pos_of_node[c],
            }
        )
    return cfg, data


def build_inputs(cfg: Cfg, x, W1, b1, W2, b2, sched):
    C, npc, npcp = cfg.C, cfg.npc, cfg.npcp
    x = np.asarray(x, dtype=np.float32)
    in_maps = []
    for c in range(C):
        xT = np.zeros((P, npcp), dtype=np.float32)
        pos = sched[c]["pos_of_node"]
        xT[:, pos] = x[c * npc : (c + 1) * npc].T
        deg = sched[c]["deg_nm"]
        dinv = (1.0 / np.sqrt(deg)).astype(np.float32)
        in_maps.append(
            {
                "xT": xT.astype(ml_dtypes.bfloat16),
                "W1in": np.asarray(W1, np.float32).astype(ml_dtypes.bfloat16),
                "W2in": np.asarray(W2, np.float32),
                "b1in": np.asarray(b1, np.float32)[None, :],
                "b2in": np.asarray(b2, np.float32)[None, :],
                "gIdx": sched[c]["gidx"],
                "dstRel": sched[c]["dst_rel"],
                "dinvNM": dinv.reshape(cfg.NG, P).T.copy(),   # [128, NG]
                "sqdQ": np.sqrt(deg).astype(np.float32)[None, :],  # [1, npcp]
            }
        )
    return in_maps


def build_nc(cfg: Cfg):
    C, DH, DOUT = cfg.C, cfg.dh, cfg.dout
    NG, NB, npcp, winrows = cfg.NG, cfg.NB, cfg.npcp, cfg.winrows
    oct_groups = cfg.oct_groups
    block_col = cfg.block_col
    chunk_info = cfg.chunk_info
    GIDXCOLS = NB * 8
    max_chunk_nb = max(nb for (_, nb, _) in chunk_info.values())
    max_cell_nb = max(cfg.nblk)

    nc = bacc.Bacc("TRN2", target_bir_lowering=False, debug=False, num_devices=C)

    xT = nc.dram_tensor("xT", [P, npcp], BF16, kind="ExternalInput").ap()
    W1in = nc.dram_tensor("W1in", [P, DH], BF16, kind="ExternalInput").ap()
    W2in = nc.dram_tensor("W2in", [DH, DOUT], F32, kind="ExternalInput").ap()
    b1in = nc.dram_tensor("b1in", [1, DH], F32, kind="ExternalInput").ap()
    b2in = nc.dram_tensor("b2in", [1, DOUT], F32, kind="ExternalInput").ap()
    gIdx = nc.dram_tensor("gIdx", [P, GIDXCOLS], I16, kind="ExternalInput").ap()
    dstRel = nc.dram_tensor("dstRel", [P, NB], BF16, kind="ExternalInput").ap()
    dinvNM = nc.dram_tensor("dinvNM", [P, NG], F32, kind="ExternalInput").ap()
    sqdQ = nc.dram_tensor("sqdQ", [1, npcp], F32, kind="ExternalInput").ap()
    z = nc.dram_tensor("z", [npcp, DOUT], F32, kind="ExternalOutput").ap()

    with tile.TileContext(nc) as tc:
        with (
            tc.tile_pool(name="const", bufs=1) as cpool,
            tc.tile_pool(name="work", bufs=1) as wpool,
            tc.tile_pool(name="psum", bufs=1, space="PSUM") as ppool,
            tc.tile_pool(name="dram", bufs=1, space="DRAM") as dpool,
        ):
            W1sb = cpool.tile([P, DH], BF16)
            nc.sync.dma_start(W1sb[:], W1in[:])
            W2sb = cpool.tile([DH, DOUT], F32)
            nc.sync.dma_start(W2sb[:], W2in[:])
            b1sb = cpool.tile([1, DH], F32)
            nc.sync.dma_start(b1sb[:], b1in[:])
            b2sb = cpool.tile([1, DOUT], F32)
            nc.sync.dma_start(b2sb[:], b2in[:])
            dinv = cpool.tile([P, NG], F32)
            nc.sync.dma_start(dinv[:], dinvNM[:])
            sqd = cpool.tile([1, npcp], F32)
            nc.sync.dma_start(sqd[:], sqdQ[:])
            dst_sb = cpool.tile([P, NB], BF16)
            nc.sync.dma_start(dst_sb[:], dstRel[:])
            gidx_sb = cpool.tile([P, GIDXCOLS], I16)
            nc.sync.dma_start(gidx_sb[:], gIdx[:])

            zrow = cpool.tile([1, 512], F32)
            nc.vector.memset(zrow[:], 0.0)
            identb = cpool.tile([P, P], BF16)
            make_identity(nc, identb[:])
            iota_i = cpool.tile([P, P], mybir.dt.int32)
            nc.gpsimd.iota(iota_i[:], pattern=[[1, P]], base=0, channel_multiplier=0)
            iota_b = cpool.tile([P, P], BF16)
            nc.vector.tensor_copy(iota_b[:], iota_i[:])

            stage1 = cpool.tile([P, NG * DH], BF16)   # G1 own rows (table vals)
            stage2 = cpool.tile([P, NG * DH], BF16)   # G2 own rows

            G1s = dpool.tile([npcp, P], BF16)
            G1f = dpool.tile([C * npcp, P], BF16, addr_space="Shared")
            G2s = dpool.tile([npcp, P], BF16)
            G2f = dpool.tile([C * npcp, P], BF16, addr_space="Shared")
            rg = [list(range(C))]

            # ---- layer 1 dense: stage1 = dinv * (x @ W1) (bf16), to G1s ----
            xt_all = cpool.tile([P, npcp], BF16)
            nc.sync.dma_start(xt_all[:], xT[:])
            for g in range(NG):
                ps = ppool.tile([P, DH], F32, tag="scratch", bufs=2, space="PSUM")
                nc.tensor.matmul(
                    ps[:], lhsT=xt_all[:, g * P : (g + 1) * P], rhs=W1sb[:],
                    start=True, stop=True,
                )
                sl = stage1[:, g * DH : (g + 1) * DH]
                nc.scalar.activation(
                    sl, ps[:], mybir.ActivationFunctionType.Copy,
                    scale=dinv[:, g : g + 1],
                )
                nc.sync.dma_start(G1s[g * P : (g + 1) * P, 0:DH], sl)

            nc.gpsimd.collective_compute(
                "AllGather", mybir.AluOpType.bypass, replica_groups=rg,
                ins=[G1s[:]], outs=[G1f[:]],
            )

            # last nonempty (w, j) per group, for the PSUM stop flag
            last_wj = {}
            for g in range(NG):
                last_wj[g] = None
                for w in range(NWIN):
                    if cfg.cell_nblk(g, w) > 0:
                        last_wj[g] = (w, cfg.cell_nblk(g, w) - 1)

            def run_layer(Gf, layer):
                for o, (g0, gn) in enumerate(oct_groups):
                    # pack group accumulators into bank-sized PSUM tiles
                    psg_of = {}
                    if layer == 1:
                        nbank = -(-gn // 8)
                        banks = [
                            ppool.tile(
                                [P, 512], F32, tag=f"ps1b_{k}", bufs=1,
                                space="PSUM", name=f"psg1_{o}_{k}",
                            )
                            for k in range(nbank)
                        ]
                        for g in range(g0, g0 + gn):
                            i = g - g0
                            psg_of[g] = banks[i // 8][:, (i % 8) * DH : (i % 8 + 1) * DH]
                        for k in range(nbank):
                            # start=True resets the whole PSUM bank on HW, so
                            # zero each bank once and accumulate into slices
                            nc.tensor.matmul(
                                banks[k][:], lhsT=zrow[:, 0:P], rhs=zrow[:],
                                start=True, stop=False, skip_group_check=True,
                            )
                    else:
                        nbank = -(-gn // 4)
                        banks = [
                            ppool.tile(
                                [DH, 512], F32, tag=f"ps2b_{k}", bufs=1,
                                space="PSUM", name=f"psg2_{o}_{k}",
                            )
                            for k in range(nbank)
                        ]
                        for g in range(g0, g0 + gn):
                            i = g - g0
                            psg_of[g] = banks[i // 4][:, (i % 4) * P : (i % 4 + 1) * P]
                        for k in range(nbank):
                            nc.tensor.matmul(
                                banks[k][:], lhsT=zrow[:, 0:DH], rhs=zrow[:],
                                start=True, stop=False, skip_group_check=True,
                            )
                    for w in range(NWIN):
                        b0, nb, c0 = chunk_info[(o, w)]
                        if nb == 0:
                            continue
                        rows = wpool.tile(
                            [P, max_chunk_nb, P], BF16, tag="rows", bufs=2
                        )
                        nc.gpsimd.dma_gather(
                            out_ap=rows[:, 0:nb, :],
                            in_ap=Gf[w * winrows : (w + 1) * winrows, :],
                            idxs_ap=gidx_sb[:, c0 : c0 + nb * 8],
                            num_idxs=nb * P,
                            num_idxs_reg=nb * P,
                            elem_size=P,
                            single_packet=False,
                        )
                        jj = 0
                        for g in range(g0, g0 + gn):
                            cnb = cfg.cell_nblk(g, w)
                            bcol = block_col[(g, w)]
                            psg = psg_of[g]
                            first_w = (w == 0) or all(
                                cfg.cell_nblk(g, w2) == 0 for w2 in range(w)
                            )
                            if first_w:
                                # self-loop + bias injection opens the group
                                only = last_wj[g] is None
                                if layer == 1:
                                    nc.tensor.matmul(
                                        psg, lhsT=identb[:],
                                        rhs=stage1[:, g * DH : (g + 1) * DH],
                                        start=False, stop=False,
                                        skip_group_check=True,
                                    )
                                    nc.tensor.matmul(
                                        psg,
                                        lhsT=sqd[:, g * P : (g + 1) * P],
                                        rhs=b1sb[:],
                                        start=False, stop=only,
                                        skip_group_check=True,
                                    )
                                else:
                                    nc.tensor.matmul(
                                        psg,
                                        lhsT=stage2[:, g * DH : (g + 1) * DH],
                                        rhs=identb[:],
                                        start=False, stop=only,
                                        skip_group_check=True,
                                    )
                            if cnb == 0:
                                continue
                            S = wpool.tile(
                                [P, max_cell_nb, P], BF16, tag="S", bufs=4
                            )
                            nc.vector.tensor_tensor(
                                out=S[:, 0:cnb, :],
                                in0=dst_sb[:, bcol : bcol + cnb]
                                .unsqueeze(2).to_broadcast([P, cnb, P]),
                                in1=iota_b[:].unsqueeze(1).to_broadcast([P, cnb, P]),
                                op=mybir.AluOpType.is_equal,
                            )
                            for j in range(cnb):
                                last = last_wj[g] == (w, j)
                                if layer == 1:
                                    nc.tensor.matmul(
                                        psg, lhsT=S[:, j, :],
                                        rhs=rows[:, jj + j, 0:DH],
                                        start=False, stop=last,
                                        skip_group_check=True,
                                    )
                                else:
                                    nc.tensor.matmul(
                                        psg, lhsT=rows[:, jj + j, 0:DH],
                                        rhs=S[:, j, :],
                                        start=False, stop=last,
                                        skip_group_check=True,
                                    )
                            jj += cnb
                    # finish the octant's groups
                    for g in range(g0, g0 + gn):
                        psg = psg_of[g]
                        if layer == 1:
                            tmp = wpool.tile([P, DH], F32, tag="f1tmp", bufs=3)
                            nc.scalar.activation(
                                tmp[:], psg, mybir.ActivationFunctionType.Relu,
                                scale=dinv[:, g : g + 1],
                            )
                            sl = stage2[:, g * DH : (g + 1) * DH]
                            nc.scalar.activation(
                                sl, tmp[:], mybir.ActivationFunctionType.Copy,
                                scale=dinv[:, g : g + 1],
                            )
                            nc.sync.dma_start(G2s[g * P : (g + 1) * P, 0:DH], sl)

                        else:
                            aT = wpool.tile([DH, P], F32, tag="aT", bufs=3)
                            nc.scalar.activation(
                                aT[:], psg, mybir.ActivationFunctionType.Copy,
                            )
                            zpt = ppool.tile(
                                [P, DH], F32, tag="scratch", bufs=2, space="PSUM"
                            )
                            zp = zpt[:, 0:DOUT]
                            nc.tensor.matmul(
                                zp, lhsT=aT[:], rhs=W2sb[:],
                                start=True, stop=False,
                            )
                            nc.tensor.matmul(
                                zp, lhsT=sqd[:, g * P : (g + 1) * P],
                                rhs=b2sb[:], start=False, stop=True,
                            )
                            zs = wpool.tile([P, DOUT], F32, tag="zs", bufs=3)
                            nc.scalar.activation(
                                zs[:], zp, mybir.ActivationFunctionType.Copy,
                                scale=dinv[:, g : g + 1],
                            )
                            nc.sync.dma_start(z[g * P : (g + 1) * P, :], zs[:])

            run_layer(G1f, 1)

            nc.gpsimd.collective_compute(
                "AllGather", mybir.AluOpType.bypass, replica_groups=rg,
                ins=[G2s[:]], outs=[G2f[:]],
            )

            run_layer(G2f, 2)

    nc.compile()
    return nc


N_CORES = 8
_NC_CACHE = {}
_SCHED_CACHE = {}


def _cached_nc(cfg):
    if cfg not in _NC_CACHE:
        _NC_CACHE[cfg] = build_nc(cfg)
    return _NC_CACHE[cfg]


def _cached_schedule(edge_index, n):
    ei = np.ascontiguousarray(edge_index)
    key = (ei.shape, n, hash(ei.tobytes()))
    if key not in _SCHED_CACHE:
        _SCHED_CACHE[key] = build_schedule(ei, n, N_CORES)
    return _SCHED_CACHE[key]


def kernel(x, W1, b1, W2, b2, edge_index):
    x = np.asarray(x)
    n = x.shape[0]
    cfg, sched = _cached_schedule(np.asarray(edge_index), n)
    in_maps = build_inputs(cfg, x, W1, b1, W2, b2, sched)
    nc = _cached_nc(cfg)
    res = bass_utils.run_bass_kernel_spmd(nc, in_maps, core_ids=list(range(N_CORES)))
    z = np.concatenate(
        [res.results[c]["z"][sched[c]["pos_of_node"]] for c in range(N_CORES)], axis=0
    )
    return z.astype(np.float32)
